# revision 1
# baseline (speedup 1.0000x reference)
"""Trainium2 Bass kernel for nn_ComplexAttention (B=8, C=512, H=W=32, HEADS=8).

Strategy
--------
Data-parallel over batch: one batch element per NeuronCore (8 cores), no
collectives.  Host-side algebraic fusion shrinks the per-core work:

  reference:  Q = R_q Wq Z,  K = R_k Wk Z,  V = R_v Wv Z   (complex, [C,T])
              S = Re(Q^H K)/sqrt(dh),  causal softmax -> A
              out = R_o Wo (V A^T)

  fused:      M = Wq^T diag(e^{i(phi_k-phi_q)}) Wk / sqrt(dh)   (host, f64)
              N = diag(e^{i phi_o}) Wo diag(e^{i phi_v}) Wv     (host, f64)
              Y = M Z            (channel-major [C,T])
              S = Re(Z^H Y)      = Zre^T Yre + Zim^T Yim
              A = softmax(causal(S))        (no max-subtraction: |S| < ~30)
              U = N Z            (token-major [T,C])
              out = U^T A^T      (channel-major [C,T], = re/im pair)

Per-core tensor-engine work is ~320 [128x128x512] matmuls + 36 transposes,
all bf16 (1 cyc/row on the PE at any N, LDWEIGHTS at half the f32 cost,
half the DMA bytes); PSUM accumulates fp32.  End-to-end rel err ~7.8e-3
against the f64 oracle (budget 2e-2).

Schedule notes (from HW traces):
 - input DMA is BW-bound, so loads are interleaved with the first matmul
   phases (mtre+zre -> Y_re, ntre -> U_re, zim -> rest) on ONE sync
   queue (a second parallel queue steals HBM bandwidth from the critical
   first loads - measured).
 - softmax exp reads scores straight out of PSUM (no copy), per-chunk
   partial row-sums are added on DVE afterwards.
 - t-tiles 4..7 are processed first so the final out chunk (t 512..1023)
   overlaps the scores/softmax of t-tiles 0..3.
"""

import math

import numpy as np

import concourse.mybir as mybir
import concourse.tile as tile
from concourse import bacc
from concourse.bass_utils import run_bass_kernel_spmd

B, C, HH, WW = 8, 512, 32, 32
T = HH * WW          # 1024 tokens
DH = C // 8          # head dim (scale only)
P = 128
CT = C // P          # 4 channel tiles
TT = T // P          # 8 token tiles
NEG = -1.0e30
DIAG_SCALE = False   # PE transpose mode requires a permutation matrix

f32 = mybir.dt.float32
f32r = mybir.dt.float32r
bf16 = mybir.dt.bfloat16
VALUE_BF16 = True    # U / P / P^T path in bf16
FULL_BF16 = True     # scores path (Z, M, N, Y) in bf16 too (7.8e-3)


def _mm(nc, out, lhsT, rhs, start, stop):
    nc.tensor.matmul(out, lhsT, rhs, start=start, stop=stop)


_CACHE: dict = {}


def _get_program(has_imag: bool):
    key = has_imag
    if key not in _CACHE:
        _CACHE[key] = _build_program(has_imag)
    return _CACHE[key]


def _build_program(has_imag: bool):
    nc = bacc.Bacc("TRN2", target_bir_lowering=False, debug=False)

    sdt = bf16 if FULL_BF16 else f32r
    zre_d = nc.dram_tensor("zre", [C, T], sdt, kind="ExternalInput").ap()
    zim_d = nc.dram_tensor("zim", [C, T], sdt, kind="ExternalInput").ap()
    mtre_d = nc.dram_tensor("mtre", [C, C], sdt, kind="ExternalInput").ap()
    ntre_d = nc.dram_tensor("ntre", [C, C], sdt, kind="ExternalInput").ap()
    if has_imag:
        mtim_d = nc.dram_tensor("mtim", [C, C], sdt, kind="ExternalInput").ap()
        mtimn_d = nc.dram_tensor("mtimn", [C, C], sdt, kind="ExternalInput").ap()
        ntim_d = nc.dram_tensor("ntim", [C, C], sdt, kind="ExternalInput").ap()
        ntimn_d = nc.dram_tensor("ntimn", [C, C], sdt, kind="ExternalInput").ap()
    vdt = bf16 if VALUE_BF16 else f32r
    ident_d = nc.dram_tensor("ident", [P, P], vdt, kind="ExternalInput").ap()
    tri_d = nc.dram_tensor("tri", [P, P], f32, kind="ExternalInput").ap()
    trif_d = nc.dram_tensor("trif", [P, 256], f32, kind="ExternalInput").ap()
    zpad_d = nc.dram_tensor("zpad", [P, 384], vdt, kind="ExternalInput").ap()
    odt = bf16 if FULL_BF16 else f32
    outre_d = nc.dram_tensor("outre", [C, T], odt, kind="ExternalOutput").ap()
    outim_d = nc.dram_tensor("outim", [C, T], odt, kind="ExternalOutput").ap()

    with tile.TileContext(nc) as tc:
        with (
            tc.tile_pool(name="const", bufs=1) as cp,
            tc.tile_pool(name="work", bufs=4) as wp,
            tc.tile_pool(name="small", bufs=12) as sp,
            tc.tile_pool(name="psmm", bufs=6, space="PSUM") as pmm,
            tc.tile_pool(name="pstr", bufs=2, space="PSUM") as ptr,
        ):
            def load_rows_on(dram, tag, eng):
                tiles = []
                for c in range(CT):
                    t = cp.tile([P, C], sdt, tag=f"{tag}{c}",
                                name=f"{tag}{c}")
                    eng.dma_start(out=t, in_=dram[c * P:(c + 1) * P, :])
                    tiles.append(t)
                return tiles

            def load_half(dram, tag, half, eng):
                tiles = []
                for c in range(CT):
                    t = cp.tile([P, 512], sdt, tag=f"{tag}{c}_{half}",
                                name=f"{tag}{c}_{half}")
                    eng.dma_start(
                        out=t,
                        in_=dram[c * P:(c + 1) * P,
                                 half * 512:(half + 1) * 512])
                    tiles.append(t)
                return tiles

            # -- small constants + first compute inputs ---------------------
            # single sync queue (HBM BW is shared; parallel queues starve
            # the critical first loads), ordered by first use, with mtre/zre
            # interleaved per c-tile so accumulation starts after ~0.5MB.
            ident = cp.tile([P, P], vdt, tag="ident", name="ident")
            nc.gpsimd.dma_start(out=ident, in_=ident_d)
            tri = cp.tile([P, P], f32, tag="tri", name="tri")
            nc.gpsimd.dma_start(out=tri, in_=tri_d)
            trif = cp.tile([P, 256], f32, tag="trif", name="trif")
            nc.gpsimd.dma_start(out=trif, in_=trif_d)
            mtre = [cp.tile([P, C], sdt, tag=f"mtre{c}", name=f"mtre{c}")
                    for c in range(CT)]
            zre_h = [[cp.tile([P, 512], sdt, tag=f"zre{c}_{h}",
                              name=f"zre{c}_{h}") for c in range(CT)]
                     for h in range(2)]

            def load_mtre_zre(c, h):
                if h == 0:
                    nc.sync.dma_start(out=mtre[c],
                                      in_=mtre_d[c * P:(c + 1) * P, :])
                nc.sync.dma_start(out=zre_h[h][c],
                                  in_=zre_d[c * P:(c + 1) * P,
                                            h * 512:(h + 1) * 512])

            # persistent result tiles (split by column half: precise deps)
            yre = [[cp.tile([P, 512], sdt, tag=f"yre{c}_{n}",
                            name=f"yre{c}_{n}") for n in range(2)]
                   for c in range(CT)]
            yim = [[cp.tile([P, 512], sdt, tag=f"yim{c}_{n}",
                            name=f"yim{c}_{n}") for n in range(2)]
                   for c in range(CT)]
            ure = [cp.tile([P, C], vdt, tag=f"ure{j}", name=f"ure{j}")
                   for j in range(TT)]
            uim = [cp.tile([P, C], vdt, tag=f"uim{j}", name=f"uim{j}")
                   for j in range(TT)]

            def psum_to_sbuf(dst_ap, src_ap):
                nc.vector.tensor_copy(out=dst_ap, in_=src_ap)

            def emit_y(dst, terms, load_hook=None):
                nterm = len(terms)
                for n in range(2):
                    pss = [pmm.tile([P, 512], f32, tag="mm", name="psmm")
                           for _ in range(CT)]
                    for t_i, (w, zh) in enumerate(terms):
                        for c in range(CT):
                            if load_hook is not None:
                                load_hook(c, n)
                            for m in range(CT):
                                _mm(nc, pss[m], w[c][:, m * P:(m + 1) * P],
                                    zh[n][c],
                                    start=(t_i == 0 and c == 0),
                                    stop=(t_i == nterm - 1 and c == CT - 1))
                    for m in range(CT):
                        psum_to_sbuf(dst[m][n], pss[m])

            def emit_u(dst, terms):
                for j in range(TT):
                    usl = slice((j % 4) * P, (j % 4 + 1) * P)
                    ps = pmm.tile([P, 512], f32, tag="mm", name="psmm")
                    nacc = len(terms) * CT
                    k = 0
                    for zh, w in terms:
                        for c in range(CT):
                            _mm(nc, ps, zh[j // 4][c][:, usl], w[c][:, :],
                                start=(k == 0), stop=(k == nacc - 1))
                            k += 1
                    psum_to_sbuf(dst[j], ps)

            # -- Y_re (needs mtre+zre only), then stream in the rest.
            # Later loads are EMITTED after emit_y so the watermark-style
            # sem waits on the first matmuls don't cover them; the DMA
            # engines still run their own streams immediately.
            if not has_imag:
                emit_y(yre, [(mtre, zre_h)], load_hook=load_mtre_zre)
                ntre = load_rows_on(ntre_d, "ntre", nc.sync)
                zim_h = [load_half(zim_d, "zim", 0, nc.sync),
                         load_half(zim_d, "zim", 1, nc.sync)]
                emit_u(ure, [(zre_h, ntre)])
                emit_y(yim, [(mtre, zim_h)])
                emit_u(uim, [(zim_h, ntre)])
            else:
                for c in range(CT):
                    load_mtre_zre(c, 0)
                for c in range(CT):
                    load_mtre_zre(c, 1)
                zim_h = [load_half(zim_d, "zim", 0, nc.sync),
                         load_half(zim_d, "zim", 1, nc.sync)]
                mtim = load_rows_on(mtim_d, "mtim", nc.sync)
                mtimn = load_rows_on(mtimn_d, "mtimn", nc.sync)
                ntre = load_rows_on(ntre_d, "ntre", nc.sync)
                ntim = load_rows_on(ntim_d, "ntim", nc.sync)
                ntimn = load_rows_on(ntimn_d, "ntimn", nc.sync)
                emit_y(yre, [(mtre, zre_h), (mtimn, zim_h)])
                emit_y(yim, [(mtre, zim_h), (mtim, zre_h)])
                emit_u(ure, [(zre_h, ntre), (zim_h, ntimn)])
                emit_u(uim, [(zim_h, ntre), (zre_h, ntim)])

            # -- P^T blocks (u-tile j, t-chunk n); zero upper regions -------
            pt = {}
            for j in range(TT):
                for n in range(2):
                    if n == 0 and j >= 4:
                        continue
                    ptile = cp.tile([P, 512], vdt, tag=f"pt{j}_{n}",
                                    name=f"pt{j}_{n}")
                    pt[(j, n)] = ptile
                    # the OUT clamp reads from col 256 even when the first
                    # transposed block starts later -> zero-fill the gap
                    lo = j * P - n * 512
                    if lo > 256:
                        nc.sync.dma_start(out=ptile[:, 256:lo],
                                          in_=zpad_d[:, 0:lo - 256])

            def emit_out_chunk(n, half=None, cols=(0, 512)):
                """out[:, n*512+cols] = U^T @ P^T for re and/or im."""
                c0, c1 = cols
                width = c1 - c0
                jmax = 4 * n + 3
                tsl = slice(n * 512 + c0, n * 512 + c1)
                pairs = ((ure, outre_d, nc.sync), (uim, outim_d, nc.sync))
                if half is not None:
                    pairs = (pairs[half],)
                js = [j for j in range(jmax + 1)
                      if max(c0, j * P - n * 512) < c1]
                for u, dram, oeng in pairs:
                    # two combined SBUF tiles -> two 512KB DMAs per half,
                    # so the transfer starts after 2 copies and the final
                    # post-compute drain is halved
                    dview = dram.rearrange("(m p) t -> p m t", p=P)
                    for mh in range(2):
                        o = wp.tile([P, 2, 512], odt, tag="osb", name="osb")
                        for mi in range(2):
                            m = 2 * mh + mi
                            msl = slice(m * P, (m + 1) * P)
                            ps = pmm.tile([P, 512], f32, tag="mm",
                                          name="psmm")
                            for j in js:
                                # pt[(j, n)] is all-zero left of column lo;
                                # clamp: N<256 f32r runs at 4 cyc/row
                                if FULL_BF16:
                                    lo = max(c0, j * P - n * 512)
                                else:
                                    lo = min(max(c0, j * P - n * 512),
                                             c1 - 256)
                                    lo = max(lo, c0)
                                _mm(nc, ps[:, lo - c0: width],
                                    u[j][:, msl], pt[(j, n)][:, lo:c1],
                                    start=(j == js[0]), stop=(j == js[-1]))
                            psum_to_sbuf(o[:, mi, :width], ps[:, :width])
                        oeng.dma_start(
                            out=dview[:, 2 * mh:2 * mh + 2, tsl],
                            in_=o[:, :, :width])

            # -- scores / softmax / transposes per t-tile -------------------
            def emit_scores_tile(i):
                ui = (i + 1) * P
                isl = slice((i % 4) * P, (i % 4 + 1) * P)
                s_sb = wp.tile([P, T], vdt, tag="s", name="s_sb")
                nchunks = (ui + 511) // 512
                lparts = []
                for q in range(nchunks):
                    w = min(512, ui - q * 512)
                    # widen 128-col chunks to 256: N<256 f32r matmuls run
                    # at 4 cyc/row, so the padded 256-col matmul is cheaper.
                    # Padded cols are masked to -inf -> exp 0.
                    wpad = w if FULL_BF16 else (
                        max(w, 256) if q == nchunks - 1 else w)
                    ps = pmm.tile([P, 512], f32, tag="mm", name="psmm")
                    k = 0
                    for zh, y in ((zre_h, yre), (zim_h, yim)):
                        for c in range(CT):
                            _mm(nc, ps[:, :wpad], zh[i // 4][c][:, isl],
                                y[c][q][:, :wpad],
                                start=(k == 0), stop=(k == 2 * CT - 1))
                            k += 1
                    last = q == nchunks - 1
                    if last:
                        fw = wpad - w + P   # frontier+pad width (128 or 256)
                        mask = tri if fw == P else trif
                        if wpad > fw:
                            # non-frontier part: exp straight from PSUM
                            lp = sp.tile([P, 1], f32, tag="lp", name="lp")
                            nc.scalar.activation(
                                out=s_sb[:, q * 512: q * 512 + wpad - fw],
                                in_=ps[:, : wpad - fw],
                                func=mybir.ActivationFunctionType.Exp,
                                accum_out=lp,
                            )
                            lparts.append(lp)
                        # frontier (+pad) cols: +mask (DVE), then exp
                        fr = sp.tile([P, 256], f32, tag="fr", name="fr")
                        nc.vector.tensor_add(out=fr[:, :fw],
                                             in0=ps[:, wpad - fw: wpad],
                                             in1=mask)
                        lp = sp.tile([P, 1], f32, tag="lp", name="lp")
                        nc.scalar.activation(
                            out=s_sb[:, ui - P: ui - P + fw],
                            in_=fr[:, :fw],
                            func=mybir.ActivationFunctionType.Exp,
                            accum_out=lp,
                        )
                        lparts.append(lp)
                    else:
                        lp = sp.tile([P, 1], f32, tag="lp", name="lp")
                        nc.scalar.activation(
                            out=s_sb[:, q * 512: q * 512 + w],
                            in_=ps[:, :w],
                            func=mybir.ActivationFunctionType.Exp,
                            accum_out=lp,
                        )
                        lparts.append(lp)

                lsum = lparts[0]
                for extra in lparts[1:]:
                    acc = sp.tile([P, 1], f32, tag="lacc", name="lacc")
                    nc.vector.tensor_add(out=acc, in0=lsum, in1=extra)
                    lsum = acc
                rl = sp.tile([P, 1], f32, tag="rl", name="rl")
                nc.vector.reciprocal(out=rl, in_=lsum)

                if DIAG_SCALE:
                    dg = sp.tile([P, P], f32r, tag="dg", name="dg")
                    nc.vector.tensor_scalar_mul(dg, ident, rl)
                    rhs = dg
                else:
                    nc.vector.tensor_scalar_mul(s_sb[:, :ui], s_sb[:, :ui],
                                                rl)
                    rhs = ident

                n = i // 4
                for j in range(i + 1):
                    pstile = ptr.tile([P, P], vdt, tag="tr", name="pstile")
                    nc.tensor.transpose(pstile, s_sb[:, j * P:(j + 1) * P],
                                        rhs)
                    nc.vector.tensor_copy(
                        out=pt[(j, n)][:, i * P - n * 512:
                                       (i + 1) * P - n * 512],
                        in_=pstile,
                    )

            for i in (4, 5, 6, 3):
                emit_scores_tile(i)
            emit_scores_tile(7)
            emit_scores_tile(2)
            emit_scores_tile(1)
            emit_out_chunk(1, half=0)
            emit_scores_tile(0)
            emit_out_chunk(1, half=1)
            emit_out_chunk(0, half=0)
            emit_out_chunk(0, half=1)

    nc.compile()
    return nc


def _prep_weights(Wq, phi_q, Wk, phi_k, Wv, phi_v, Wo, phi_o):
    Wq, Wk, Wv, Wo = (np.asarray(w, np.float64) for w in (Wq, Wk, Wv, Wo))
    pq, pk, pv, po = (np.asarray(p, np.float64)
                      for p in (phi_q, phi_k, phi_v, phi_o))
    M = (Wq.T @ (np.exp(1j * (pk - pq))[:, None] * Wk)) / math.sqrt(DH)
    N = (np.exp(1j * po)[:, None] * Wo) @ (np.exp(1j * pv)[:, None] * Wv)
    has_imag = not (np.allclose(M.imag, 0.0) and np.allclose(N.imag, 0.0))
    return M, N, has_imag


def _consts(has_imag, M, N):
    import ml_dtypes
    snp = ml_dtypes.bfloat16 if FULL_BF16 else np.float32
    vnp = ml_dtypes.bfloat16 if VALUE_BF16 else np.float32
    consts = {
        "mtre": np.ascontiguousarray(M.real.T.astype(snp)),
        "ntre": np.ascontiguousarray(N.real.T.astype(snp)),
        "ident": np.eye(P, dtype=vnp),
        "tri": np.triu(np.full((P, P), NEG, np.float32), 1),
        "trif": np.concatenate(
            [np.triu(np.full((P, P), NEG, np.float32), 1),
             np.full((P, P), NEG, np.float32)], axis=1),
        "zpad": np.zeros((P, 384), vnp),
    }
    if has_imag:
        mtim = np.ascontiguousarray(M.imag.T.astype(snp))
        ntim = np.ascontiguousarray(N.imag.T.astype(snp))
        consts.update(mtim=mtim, mtimn=-mtim, ntim=ntim, ntimn=-ntim)
    return consts


def kernel(z_re, z_im, Wq, phi_q, Wk, phi_k, Wv, phi_v, Wo, phi_o):
    import ml_dtypes
    snp = ml_dtypes.bfloat16 if FULL_BF16 else np.float32
    z_re = np.ascontiguousarray(np.asarray(z_re, np.float32).astype(snp))
    z_im = np.ascontiguousarray(np.asarray(z_im, np.float32).astype(snp))
    M, N, has_imag = _prep_weights(Wq, phi_q, Wk, phi_k, Wv, phi_v, Wo, phi_o)
    consts = _consts(has_imag, M, N)

    nc = _get_program(has_imag)
    in_maps = [
        dict(consts, zre=z_re[b].reshape(C, T), zim=z_im[b].reshape(C, T))
        for b in range(B)
    ]
    res = run_bass_kernel_spmd(nc, in_maps, list(range(B)))
    out_re = np.stack([np.asarray(res.results[b]["outre"], np.float32)
                       .reshape(C, HH, WW) for b in range(B)])
    out_im = np.stack([np.asarray(res.results[b]["outim"], np.float32)
                       .reshape(C, HH, WW) for b in range(B)])
    return out_re, out_im



# revision 3
# speedup vs baseline: 1.0332x; 1.0332x over previous
"""Trainium2 Bass kernel for nn_ComplexAttention (B=8, C=512, H=W=32, HEADS=8).

Strategy
--------
Data-parallel over batch: one batch element per NeuronCore (8 cores), no
collectives.  Host-side algebraic fusion shrinks the per-core work:

  reference:  Q = R_q Wq Z,  K = R_k Wk Z,  V = R_v Wv Z   (complex, [C,T])
              S = Re(Q^H K)/sqrt(dh),  causal softmax -> A
              out = R_o Wo (V A^T)
  fused:      M = Wq^T diag(e^{i(phi_k-phi_q)}) Wk / sqrt(dh)   (host, f64)
              N = diag(e^{i phi_o}) Wo diag(e^{i phi_v}) Wv     (host, f64)
              Y = M Z            (channel-major [C,T])
              S = Re(Z^H Y)
              E = exp(causal(S)) (no max-subtraction: |S| < ~30)
              U = N Z            (token-major [T,C])
              out[t] = (E @ U)[t] / L[t],  L = row sums of E

Everything on-device is bf16 matmul / f32 PSUM.  End-to-end rel err
~7.9e-3 against the f64 oracle (budget 2e-2).

Schedule notes (from HW traces of the previous revision):
 - scores are computed TRANSPOSED (S^T[u,t] blocks, stationary = Y
   u-slice, streaming = Z) so the exp tiles are directly usable as the
   stationary operand of the attention-out matmuls -> no PE transposes,
   no DVE transpose copies.
 - attention out is TOKEN-major [t, c]; softmax normalization is a
   per-partition tensor_scalar during the PSUM->SBUF copy (free), with
   row sums L accumulated by N=1 matmuls against a ones vector that
   share the stationary weights of the out matmuls.  The host undoes
   the token-major layout during unsharding.
 - input DMA is spread across three queues (sync/scalar/gpsimd) in
   consumption order; one queue alone feeds ~200 GB/s which stalled
   the PE in the previous revision.
 - a memset tile + 6 dummy matmuls at the head of the PE queue keep
   the PE_HAM activity monitor busy during the DMA lead-in so the real
   matmul stream starts at 2.4 GHz instead of 1.2 GHz.
 - S^T block j is emitted one step ahead of out(j-1) so exp (scalar
   engine) always overlaps matmuls.
"""

import math

import numpy as np

import concourse.mybir as mybir
import concourse.tile as tile
from concourse import bacc
from concourse.bass_utils import run_bass_kernel_spmd

B, C, HH, WW = 8, 512, 32, 32
T = HH * WW          # 1024 tokens
DH = C // 8          # head dim (scale only)
P = 128
CT = C // P          # 4 channel tiles
TT = T // P          # 8 token tiles
NEG = -1.0e30
NDUMMY = 6

f32 = mybir.dt.float32
bf16 = mybir.dt.bfloat16


def _mm(nc, out, lhsT, rhs, start, stop):
    nc.tensor.matmul(out, lhsT, rhs, start=start, stop=stop)


_CACHE: dict = {}


def _get_program(has_imag: bool):
    key = has_imag
    if key not in _CACHE:
        _CACHE[key] = _build_program(has_imag)
    return _CACHE[key]


def _build_program(has_imag: bool):
    nc = bacc.Bacc("TRN2", target_bir_lowering=False, debug=False)

    zre_d = nc.dram_tensor("zre", [C, T], bf16, kind="ExternalInput").ap()
    zim_d = nc.dram_tensor("zim", [C, T], bf16, kind="ExternalInput").ap()
    mtre_d = nc.dram_tensor("mtre", [C, C], bf16, kind="ExternalInput").ap()
    ntre_d = nc.dram_tensor("ntre", [C, C], bf16, kind="ExternalInput").ap()
    if has_imag:
        mtim_d = nc.dram_tensor("mtim", [C, C], bf16, kind="ExternalInput").ap()
        mtimn_d = nc.dram_tensor("mtimn", [C, C], bf16, kind="ExternalInput").ap()
        ntim_d = nc.dram_tensor("ntim", [C, C], bf16, kind="ExternalInput").ap()
        ntimn_d = nc.dram_tensor("ntimn", [C, C], bf16, kind="ExternalInput").ap()
    trit_d = nc.dram_tensor("trit", [P, P], f32, kind="ExternalInput").ap()
    # token-major [T, C] outputs; the host transposes while unsharding
    outre_d = nc.dram_tensor("outre", [T, C], bf16, kind="ExternalOutput").ap()
    outim_d = nc.dram_tensor("outim", [T, C], bf16, kind="ExternalOutput").ap()

    with tile.TileContext(nc) as tc:
        with (
            tc.tile_pool(name="const", bufs=1) as cp,
            tc.tile_pool(name="work", bufs=4) as wp,
            tc.tile_pool(name="small", bufs=12) as sp,
            tc.tile_pool(name="psmm", bufs=6, space="PSUM") as pmm,
            tc.tile_pool(name="psl", bufs=2, space="PSUM") as psl,
        ):
            # -- constants (no DMA except the mask) + HAM warmup ------------
            dum = cp.tile([P, 512], bf16, tag="dum", name="dum")
            nc.gpsimd.memset(dum, 0.0)
            ones = cp.tile([P, 1], bf16, tag="ones", name="ones")
            nc.gpsimd.memset(ones, 1.0)
            trit = cp.tile([P, P], f32, tag="trit", name="trit")
            nc.gpsimd.dma_start(out=trit, in_=trit_d)

            pdum = pmm.tile([P, 512], f32, tag="mm", name="pdum")
            for _ in range(NDUMMY):
                nc.tensor.matmul(pdum, dum[:, :P], dum, start=True, stop=True)

            # -- persistent input tiles -------------------------------------
            mtre = [cp.tile([P, C], bf16, tag=f"mtre{c}", name=f"mtre{c}")
                    for c in range(CT)]
            ntre = [cp.tile([P, C], bf16, tag=f"ntre{c}", name=f"ntre{c}")
                    for c in range(CT)]
            zre_h = [[cp.tile([P, 512], bf16, tag=f"zre{c}_{h}",
                              name=f"zre{c}_{h}") for c in range(CT)]
                     for h in range(2)]
            zim_h = [[cp.tile([P, 512], bf16, tag=f"zim{c}_{h}",
                              name=f"zim{c}_{h}") for c in range(CT)]
                     for h in range(2)]

            def ld(eng, t, dram, r0, c0, w):
                eng.dma_start(out=t, in_=dram[r0:r0 + P, c0:c0 + w])

            # loads in consumption order, split across three queues so the
            # aggregate feed (~2x200 GB/s + zim on gpsimd) outruns the PE.
            ld(nc.sync, mtre[0], mtre_d, 0, 0, C)
            ld(nc.scalar, zre_h[0][0], zre_d, 0, 0, 512)
            ld(nc.scalar, mtre[1], mtre_d, P, 0, C)
            ld(nc.sync, zre_h[0][1], zre_d, P, 0, 512)
            ld(nc.sync, mtre[2], mtre_d, 2 * P, 0, C)
            ld(nc.scalar, zre_h[0][2], zre_d, 2 * P, 0, 512)
            ld(nc.scalar, mtre[3], mtre_d, 3 * P, 0, C)
            ld(nc.sync, zre_h[0][3], zre_d, 3 * P, 0, 512)
            ld(nc.scalar, zre_h[1][0], zre_d, 0, 512, 512)
            ld(nc.sync, zre_h[1][1], zre_d, P, 512, 512)
            ld(nc.scalar, zre_h[1][2], zre_d, 2 * P, 512, 512)
            ld(nc.sync, zre_h[1][3], zre_d, 3 * P, 512, 512)
            for c in range(CT):
                ld((nc.scalar, nc.sync)[c % 2], ntre[c], ntre_d, c * P, 0, C)
            for h in range(2):
                for c in range(CT):
                    ld(nc.gpsimd, zim_h[h][c], zim_d, c * P, h * 512, 512)
            if has_imag:
                mtim = [cp.tile([P, C], bf16, tag=f"mtim{c}") for c in range(CT)]
                mtimn = [cp.tile([P, C], bf16, tag=f"mtimn{c}") for c in range(CT)]
                ntim = [cp.tile([P, C], bf16, tag=f"ntim{c}") for c in range(CT)]
                ntimn = [cp.tile([P, C], bf16, tag=f"ntimn{c}") for c in range(CT)]
                for c in range(CT):
                    ld(nc.sync, mtimn[c], mtimn_d, c * P, 0, C)
                    ld(nc.scalar, mtim[c], mtim_d, c * P, 0, C)
                    ld(nc.sync, ntim[c], ntim_d, c * P, 0, C)
                    ld(nc.scalar, ntimn[c], ntimn_d, c * P, 0, C)

            # -- Y = M Z (channel-major), U = N Z (token-major) -------------
            yre = [[cp.tile([P, 512], bf16, tag=f"yre{c}_{n}",
                            name=f"yre{c}_{n}") for n in range(2)]
                   for c in range(CT)]
            yim = [[cp.tile([P, 512], bf16, tag=f"yim{c}_{n}",
                            name=f"yim{c}_{n}") for n in range(2)]
                   for c in range(CT)]
            ure = [cp.tile([P, C], bf16, tag=f"ure{j}", name=f"ure{j}")
                   for j in range(TT)]
            uim = [cp.tile([P, C], bf16, tag=f"uim{j}", name=f"uim{j}")
                   for j in range(TT)]

            def emit_y(dst, terms):
                nterm = len(terms)
                for n in range(2):
                    pss = [pmm.tile([P, 512], f32, tag="mm", name="psmm")
                           for _ in range(CT)]
                    for t_i, (w, zh) in enumerate(terms):
                        for c in range(CT):
                            for m in range(CT):
                                _mm(nc, pss[m], w[c][:, m * P:(m + 1) * P],
                                    zh[n][c],
                                    start=(t_i == 0 and c == 0),
                                    stop=(t_i == nterm - 1 and c == CT - 1))
                    for m in range(CT):
                        nc.vector.tensor_copy(out=dst[m][n], in_=pss[m])

            def emit_u(dst, terms):
                for j in range(TT):
                    usl = slice((j % 4) * P, (j % 4 + 1) * P)
                    ps = pmm.tile([P, 512], f32, tag="mm", name="psmm")
                    nacc = len(terms) * CT
                    k = 0
                    for zh, w in terms:
                        for c in range(CT):
                            _mm(nc, ps, zh[j // 4][c][:, usl], w[c][:, :],
                                start=(k == 0), stop=(k == nacc - 1))
                            k += 1
                    nc.vector.tensor_copy(out=dst[j], in_=ps)

            if not has_imag:
                emit_y(yre, [(mtre, zre_h)])
                emit_u(ure, [(zre_h, ntre)])
                emit_y(yim, [(mtre, zim_h)])
                emit_u(uim, [(zim_h, ntre)])
            else:
                emit_y(yre, [(mtre, zre_h), (mtimn, zim_h)])
                emit_u(ure, [(zre_h, ntre), (zim_h, ntimn)])
                emit_y(yim, [(mtre, zim_h), (mtim, zre_h)])
                emit_u(uim, [(zim_h, ntre), (zre_h, ntim)])

            # -- transposed scores S^T[u,t] -> exp tiles E^T ----------------
            # block j covers u in [128j, 128j+128), t in [128j, 1024)
            sT = [cp.tile([P, T - j * P], bf16, tag=f"sT{j}", name=f"sT{j}")
                  for j in range(TT)]

            def emit_scores_block(j):
                base = j * P
                if j < 4:
                    chunks = [(base, 512), (512, 1024)]
                else:
                    chunks = [(base, 1024)]
                for a, b in chunks:
                    w = b - a
                    ps = pmm.tile([P, 512], f32, tag="mm", name="psmm")
                    k = 0
                    for zh, y in ((zre_h, yre), (zim_h, yim)):
                        for c in range(CT):
                            lhsT = y[c][j // 4][:, (j % 4) * P:(j % 4 + 1) * P]
                            rhs = zh[a // 512][c][:, a % 512:a % 512 + w]
                            _mm(nc, ps[:, :w], lhsT, rhs,
                                start=(k == 0), stop=(k == 2 * CT - 1))
                            k += 1
                    loc = a - base
                    if loc == 0:
                        # causal diagonal block: mask(+exp) via fr staging
                        fr = sp.tile([P, P], f32, tag="fr", name="fr")
                        nc.vector.tensor_add(out=fr, in0=ps[:, :P], in1=trit)
                        nc.scalar.activation(
                            out=sT[j][:, 0:P], in_=fr,
                            func=mybir.ActivationFunctionType.Exp)
                        if w > P:
                            nc.scalar.activation(
                                out=sT[j][:, P:w], in_=ps[:, P:w],
                                func=mybir.ActivationFunctionType.Exp)
                    else:
                        nc.scalar.activation(
                            out=sT[j][:, loc:loc + w], in_=ps[:, :w],
                            func=mybir.ActivationFunctionType.Exp)

            # -- attention out, token-major, normalization fused ------------
            store_eng = [nc.sync, nc.scalar, nc.gpsimd]

            def emit_out(i):
                ps_re = pmm.tile([P, 512], f32, tag="mm", name="psmm")
                ps_im = pmm.tile([P, 512], f32, tag="mm", name="psmm")
                ps_l = psl.tile([P, 1], f32, tag="l", name="psl")
                for j in range(i + 1):
                    loc = (i - j) * P
                    lhsT = sT[j][:, loc:loc + P]
                    _mm(nc, ps_re, lhsT, ure[j], start=(j == 0), stop=(j == i))
                    _mm(nc, ps_im, lhsT, uim[j], start=(j == 0), stop=(j == i))
                    _mm(nc, ps_l, lhsT, ones, start=(j == 0), stop=(j == i))
                rl = sp.tile([P, 1], f32, tag="rl", name="rl")
                nc.vector.reciprocal(out=rl, in_=ps_l)
                for half, (ps, dram) in enumerate(
                        ((ps_re, outre_d), (ps_im, outim_d))):
                    o = wp.tile([P, 512], bf16, tag="osb", name="osb")
                    nc.vector.tensor_scalar_mul(o, ps, rl)
                    store_eng[(2 * i + half) % 3].dma_start(
                        out=dram[i * P:(i + 1) * P, :], in_=o)

            emit_scores_block(0)
            for j in range(1, TT):
                emit_scores_block(j)
                emit_out(j - 1)
            emit_out(TT - 1)

    nc.compile()
    return nc


def _prep_weights(Wq, phi_q, Wk, phi_k, Wv, phi_v, Wo, phi_o):
    Wq, Wk, Wv, Wo = (np.asarray(w, np.float64) for w in (Wq, Wk, Wv, Wo))
    pq, pk, pv, po = (np.asarray(p, np.float64)
                      for p in (phi_q, phi_k, phi_v, phi_o))
    M = (Wq.T @ (np.exp(1j * (pk - pq))[:, None] * Wk)) / math.sqrt(DH)
    N = (np.exp(1j * po)[:, None] * Wo) @ (np.exp(1j * pv)[:, None] * Wv)
    has_imag = not (np.allclose(M.imag, 0.0) and np.allclose(N.imag, 0.0))
    return M, N, has_imag


def _consts(has_imag, M, N):
    import ml_dtypes
    snp = ml_dtypes.bfloat16
    consts = {
        "mtre": np.ascontiguousarray(M.real.T.astype(snp)),
        "ntre": np.ascontiguousarray(N.real.T.astype(snp)),
        # S^T diag-block causal mask: -inf where u > t (rows > cols)
        "trit": np.tril(np.full((P, P), NEG, np.float32), -1),
    }
    if has_imag:
        mtim = np.ascontiguousarray(M.imag.T.astype(snp))
        ntim = np.ascontiguousarray(N.imag.T.astype(snp))
        consts.update(mtim=mtim, mtimn=-mtim, ntim=ntim, ntimn=-ntim)
    return consts


def kernel(z_re, z_im, Wq, phi_q, Wk, phi_k, Wv, phi_v, Wo, phi_o):
    import ml_dtypes
    snp = ml_dtypes.bfloat16
    z_re = np.ascontiguousarray(np.asarray(z_re, np.float32).astype(snp))
    z_im = np.ascontiguousarray(np.asarray(z_im, np.float32).astype(snp))
    M, N, has_imag = _prep_weights(Wq, phi_q, Wk, phi_k, Wv, phi_v, Wo, phi_o)
    consts = _consts(has_imag, M, N)

    nc = _get_program(has_imag)
    in_maps = [
        dict(consts, zre=z_re[b].reshape(C, T), zim=z_im[b].reshape(C, T))
        for b in range(B)
    ]
    res = run_bass_kernel_spmd(nc, in_maps, list(range(B)))
    # device output is token-major [T, C]; transpose while unsharding
    out_re = np.stack([np.asarray(res.results[b]["outre"], np.float32)
                       .reshape(T, C).T.reshape(C, HH, WW) for b in range(B)])
    out_im = np.stack([np.asarray(res.results[b]["outim"], np.float32)
                       .reshape(T, C).T.reshape(C, HH, WW) for b in range(B)])
    return out_re, out_im
